# revision 1
# baseline (speedup 1.0000x reference)
"""TRN2 Bass kernel for nn_NNModelEx_63513976373928.

Math (per row x of X [B, 38]):
  h1  = relu(x @ W1.T + b1)                  [256]
  h2  = relu(h1 @ W2.T + b2)                 [256]
  out = h2 @ W3.T + b3                       [128]
  per target t in (incl, ecc, mm), ridx in (7, 9, 12):
    lin = out . lw_t + lb_t
    e   = (out . bew_t) * eps + beb_t        eps = x[0]
    y_t = bw_t * e * lin + bb_t + x[ridx]

Device strategy (pure data parallel, 8 cores x 32768 rows):
  - feature-on-partition layout: H1T/H2T [units, rows], rows chunked by 512
  - b1 folded into the L1 matmul via an augmented contraction row
    (XT row 38 = 1, W1T row 38 = b1) -> single merged L1 relu cast
  - L3 + heads folded: lin/p are dots of h2 with W3.T @ lw / W3.T @ bew
    (6 vectors precomputed on host, b3 contributions folded into consts)
  - X pre-transposed and bf16-cast on host -> XT [39, 32768] per core
  - residual/eps/bias columns packed fp32 on host -> XRB [128, 64, 4, 9]
  - head elementwise split DVE (PSUM reads) / Pool (SBUF-only ops)
"""

import sys

for _p in ("/opt/trn_rl_repo", "/opt/trn_rl_repo/concourse"):
    if _p not in sys.path:
        sys.path.insert(0, _p)

import numpy as np
import ml_dtypes

BF16 = ml_dtypes.bfloat16

NCORES = 8
B = 262144
D = 38
DA = 39                     # contraction with bias row appended
ROWS = B // NCORES          # 32768 rows per core
CHUNK = 512                 # rows per chunk
NCHUNK = ROWS // CHUNK      # 64
CPB = 8                     # chunks per staging batch
NBATCH = NCHUNK // CPB      # 8

_NC_CACHE = {}


def _build_nc(repeat=1):
    from concourse import bass, bacc, tile
    from contextlib import nullcontext

    mybir = bass.mybir
    f32 = mybir.dt.float32
    f8 = mybir.dt.float8e4

    nc = bacc.Bacc(None, target_bir_lowering=False, debug=False)

    XT = nc.dram_tensor("XT", [20, 2, ROWS], f8, kind="ExternalInput")
    XRB = nc.dram_tensor("XRB", [128, NCHUNK, 4, 9], f32, kind="ExternalInput")
    W1T = nc.dram_tensor("W1T", [20, 2, 256], f8, kind="ExternalInput")
    # W2H packs W2 (cols 0:256, two output halves) + head vectors (256:262);
    # padded to 272 so the dim-1 stride is 16B-aligned (dual-row fp8 ISA rule)
    W2H = nc.dram_tensor("W2H", [128, 2, 272], f8, kind="ExternalInput")
    # CB packs lin consts ([:, 0:4, :]) + b2 halves ([:, 4, 0:2])
    CB = nc.dram_tensor("CB", [128, 5, 3], f32, kind="ExternalInput")
    Y = nc.dram_tensor("Y", [128, NCHUNK, 4, 3], f32, kind="ExternalOutput")

    with tile.TileContext(nc) as tc:
        with (
            tc.tile_pool(name="wpool", bufs=1) as wpool,
            tc.tile_pool(name="xpool", bufs=2) as xpool,
            tc.tile_pool(name="h1pool", bufs=3) as h1pool,
            tc.tile_pool(name="h2pool", bufs=4) as h2pool,
            tc.tile_pool(name="spool", bufs=2) as spool,
            tc.tile_pool(name="bpool", bufs=3) as bpool,
            tc.tile_pool(name="psl1", bufs=2, space="PSUM") as psl1,
            tc.tile_pool(name="psl2", bufs=1, space="PSUM") as psl2,
            tc.tile_pool(name="pshead", bufs=2, space="PSUM") as pshead,
        ):
            w1t = wpool.tile([20, 2, 256], f8)
            nc.sync.dma_start(w1t[:], W1T[:])
            w2h = wpool.tile([128, 2, 272], f8)
            nc.sync.dma_start(w2h[:], W2H[:])
            cb = wpool.tile([128, 5, 3], f32)
            nc.sync.dma_start(cb[:], CB[:])

            rep_ctx = tc.For_i(0, repeat) if repeat > 1 else nullcontext()
            with rep_ctx:
                _kernel_body(nc, tc, locals())

    nc.finalize()
    return nc


def _kernel_body(nc, tc, env):
    from concourse import bass

    mybir = bass.mybir
    f32 = mybir.dt.float32
    f8 = mybir.dt.float8e4
    DR = mybir.MatmulPerfMode.DoubleRow
    Relu = mybir.ActivationFunctionType.Relu
    add = mybir.AluOpType.add
    mult = mybir.AluOpType.mult
    amax = mybir.AluOpType.max
    TT = nc.vector.tensor_tensor
    PTT = nc.gpsimd.tensor_tensor
    XT, XRB, Y = env["XT"], env["XRB"], env["Y"]
    w1t, w2h, cb = env["w1t"], env["w2h"], env["cb"]
    xpool, h1pool, h2pool, spool, bpool = (
        env["xpool"], env["h1pool"], env["h2pool"], env["spool"],
        env["bpool"])
    psl1, psl2, pshead = env["psl1"], env["psl2"], env["pshead"]

    xrb_t = [None] * NBATCH
    xt_t = [None] * NBATCH
    hp_t = [None] * NBATCH
    h1_t = [None] * NCHUNK
    h2_t = [None] * NCHUNK

    def stage_in(bi):
        base = bi * CPB * CHUNK
        xt = xpool.tile([20, 2, CPB * CHUNK], f8, name="xt", bufs=2)
        if bi == 0:
            # split so chunk 0/1 land early and shorten the pipeline fill
            for lo, hi in ((0, CHUNK), (CHUNK, 2 * CHUNK),
                           (2 * CHUNK, CPB * CHUNK)):
                nc.sync.dma_start(xt[:, :, lo:hi],
                                  XT[:, :, base + lo:base + hi])
        else:
            nc.sync.dma_start(
                xt[:], XT[:, :, base:base + CPB * CHUNK])
        xt_t[bi] = xt
        xrb = bpool.tile([128, CPB, 4, 9], f32, name="xrb", bufs=3)
        nc.sync.dma_start(xrb[:], XRB[:, bi * CPB:(bi + 1) * CPB, :, :])
        xrb_t[bi] = xrb

    def epilogue(bi, off, n, suf):
        # y = ((p*bweps + ebias) * (lin + lb')) + (xr + bb)
        #   xrb cols: 0:3 bw*eps/SP^2, 3:6 xr+bb, 6:9 ebias/SP
        hp = hp_t[bi]
        xrb = xrb_t[bi]
        hs = slice(off, off + n)
        cb_lin = cb[:, None, 0:4, :].to_broadcast([128, n, 4, 3])
        linp = spool.tile([128, n, 4, 3], f32, name="linp" + suf, bufs=2)
        e = spool.tile([128, n, 4, 3], f32, name="e" + suf, bufs=2)
        ystg = bpool.tile([128, n, 4, 3], f32, name="ystg" + suf, bufs=2)
        TT(out=linp[:], in0=hp[:, hs, :, 0:3], in1=cb_lin, op=add)
        TT(out=e[:], in0=hp[:, hs, :, 3:6], in1=xrb[:, hs, :, 0:3],
           op=mult)
        PTT(out=e[:], in0=e[:], in1=xrb[:, hs, :, 6:9], op=add)
        PTT(out=e[:], in0=e[:], in1=linp[:], op=mult)
        PTT(out=ystg[:], in0=e[:], in1=xrb[:, hs, :, 3:6], op=add)
        nc.sync.dma_start(
            Y[:, bi * CPB + off:bi * CPB + off + n, :, :], ystg[:])

    # 4-stage software pipeline over chunks so PE never waits on casts:
    # iteration ci emits on PE [headMM(ci-3), L1MM(ci), L2MM(ci-1)].
    # Casts for chunk c run on ACT/DVE during iteration c+1 and finish
    # early in c+2; head MMs consume them one period later at c+3.
    # PSUM budget (8 banks): h1p 2x2 + h2pa 1 + h2pb 1 + hp 2x1.
    stage_in(0)
    for ci in range(NCHUNK + 3):
        ck = ci - 3
        if ck >= 0:
            bi, cbk = divmod(ck, CPB)
            if cbk == 0:
                hp_t[bi] = pshead.tile([128, CPB, 4, 6], f32, name="hp",
                                       bufs=2)
            hp = hp_t[bi]
            h2ab = h2_t[ck]
            h2_t[ck] = None
            # heads: hp[:, cbk, s, 0:3] = lin_mm, hp[:, cbk, s, 3:6] = p_mm
            for s in range(4):
                seg = slice(s * 128, (s + 1) * 128)
                nc.tensor.matmul(hp[:, cbk, s, :], h2ab[:, :, seg],
                                 w2h[:, :, 256:262], start=True, stop=True,
                                 perf_mode=DR)

        if ci < NCHUNK:
            bi, cbk = divmod(ci, CPB)
            if cbk == 0 and bi + 1 < NBATCH:
                stage_in(bi + 1)
            # L1: H1T = W1T.T @ XT, bias via augmented row, merged cast
            # weights host-scaled x64 for fp8; descaled via the ACT port
            h1p = psl1.tile([128, 2, CHUNK], f32, name="h1p", bufs=2)
            xt = xt_t[bi]
            sl = slice(cbk * CHUNK, (cbk + 1) * CHUNK)
            nc.tensor.matmul(h1p[:, 0, :], w1t[:, :, 0:128], xt[:, :, sl],
                             start=True, stop=True, perf_mode=DR)
            nc.tensor.matmul(h1p[:, 1, :], w1t[:, :, 128:256], xt[:, :, sl],
                             start=True, stop=True, perf_mode=DR)
            h1 = h1pool.tile([128, 2, CHUNK], f8, name="h1", bufs=3)
            nc.scalar.activation(h1[:], h1p[:], Relu, bias=0.0,
                                 scale=1.0 / 64)
            h1_t[ci] = h1

        cj = ci - 1
        if 0 <= cj < NCHUNK:
            # L2: H2T = W2T.T @ H1T, single DoubleRow matmul per half
            h1 = h1_t[cj]
            h1_t[cj] = None
            h2pa = psl2.tile([128, CHUNK], f32, name="h2pa", bufs=1)
            h2pb = psl2.tile([128, CHUNK], f32, name="h2pb", bufs=1)
            nc.tensor.matmul(h2pa[:], w2h[:, :, 0:128], h1[:],
                             start=True, stop=True, perf_mode=DR)
            nc.tensor.matmul(h2pb[:], w2h[:, :, 128:256], h1[:],
                             start=True, stop=True, perf_mode=DR)
            # W2/B2 host-scaled x16 -> casts produce 16*h2 in fp8
            # (head vectors carry the matching descale)
            h2ab = h2pool.tile([128, 2, CHUNK], f8, name="h2ab", bufs=4)
            # a always on DVE (first, so its PSUM slot frees early);
            # b moves to ACT 2 chunks in 7 to balance engine load
            nc.vector.tensor_scalar(h2ab[:, 0, :], h2pa[:], cb[:, 4, 0:1],
                                    0.0, op0=add, op1=amax)
            if cj % 7 in (2, 5):
                nc.scalar.activation(h2ab[:, 1, :], h2pb[:], Relu,
                                     bias=cb[:, 4, 1:2], scale=1.0)
            else:
                nc.vector.tensor_scalar(h2ab[:, 1, :], h2pb[:],
                                        cb[:, 4, 1:2], 0.0,
                                        op0=add, op1=amax)
            h2_t[cj] = h2ab

        # epilogue, emitted last so DVE runs casts first; the final batch
        # is split into two half-batches to shorten the pipeline drain
        if ck >= 0:
            if ck == NCHUNK - 5:
                epilogue(NBATCH - 1, 0, 4, "q")
            elif ck == NCHUNK - 1:
                epilogue(NBATCH - 1, 4, 4, "q")
            elif ck % CPB == CPB - 1:
                epilogue(ck // CPB, 0, CPB, "")


def _get_nc():
    if "nc" not in _NC_CACHE:
        _NC_CACHE["nc"] = _build_nc()
    return _NC_CACHE["nc"]


def _prepare_inputs(inputs):
    X = np.asarray(inputs["X"], dtype=np.float32)
    W1 = np.asarray(inputs["W1"], dtype=np.float32)
    b1 = np.asarray(inputs["b1"], dtype=np.float32)
    W2 = np.asarray(inputs["W2"], dtype=np.float32)
    b2 = np.asarray(inputs["b2"], dtype=np.float32)
    W3 = np.asarray(inputs["W3"], dtype=np.float32)
    b3 = np.asarray(inputs["b3"], dtype=np.float32)

    lw, lb, bew, beb, bw, bb = {}, {}, {}, {}, {}, {}
    for t in ("incl", "ecc", "mm"):
        lw[t] = np.asarray(inputs[f"lin_w_{t}"], np.float32)[0]        # [128]
        lb[t] = float(np.asarray(inputs[f"lin_b_{t}"], np.float32)[0])
        bew[t] = np.asarray(inputs[f"bile_w_{t}"], np.float32)[0][:, 0]  # [128]
        beb[t] = float(np.asarray(inputs[f"bile_b_{t}"], np.float32)[0])
        bw[t] = float(np.asarray(inputs[f"bil_w_{t}"], np.float32)[0, 0, 0])
        bb[t] = float(np.asarray(inputs[f"bil_b_{t}"], np.float32)[0])
    TS = ("incl", "ecc", "mm")
    RIDX = {"incl": 7, "ecc": 9, "mm": 12}

    # ---- replicated weights (fp8 DoubleRow layouts) ----
    # scales: W1 x64 (descaled in h1 ACT cast), W2/B2 x16 (h2 lives at
    # 16x in fp8, max |h2|<15 assumed), HW2 x16 -> hp at 256x; the 1/256
    # descale is folded exactly (powers of 2) into CONSTS/XRB.
    F8 = ml_dtypes.float8_e4m3
    SC1, SC2, SCH = 64.0, 16.0, 16.0
    SP = SC2 * SCH                                                  # 256
    W1a = np.zeros((40, 256), np.float32)
    W1a[0:D] = W1.T * SC1
    W1a[D] = b1 * SC1
    W1T = np.ascontiguousarray(W1a.reshape(20, 2, 256)).astype(F8)
    O6 = np.stack([lw[t] for t in TS] + [bew[t] for t in TS], axis=1)  # [128,6]
    HW2f = W3.T.astype(np.float32) @ O6                             # [256, 6]
    W2H = np.zeros((128, 2, 272), np.float32)
    # cols 0:256: W2.T [k, m] at [k % 128, k // 128, oh*128 + m], x16
    W2H[:, :, 0:256] = (
        W2.T.reshape(2, 128, 256).transpose(1, 0, 2)) * SC2
    W2H[:, :, 256:262] = (
        HW2f.reshape(2, 128, 6).transpose(1, 0, 2)) * SCH
    W2H = W2H.astype(F8)
    c3 = np.array(
        [lb[t] + float(b3 @ lw[t]) for t in TS],         # lb' (b3 folded)
        dtype=np.float32) * SP
    CB = np.empty((128, 5, 3), np.float32)
    CB[:, 0:4, :] = c3
    CB[:, 4, 0] = b2[0:128] * SC2
    CB[:, 4, 1] = b2[128:256] * SC2
    CB[:, 4, 2] = 0.0
    K = {t: float(b3 @ bew[t]) for t in TS}

    in_maps = []
    for c in range(NCORES):
        Xl = X[c * ROWS:(c + 1) * ROWS]                             # [32768, 38]
        XTf = np.zeros((40, ROWS), np.float32)
        XTf[0:D] = Xl.T
        XTf[D] = 1.0
        XTc = np.ascontiguousarray(XTf.reshape(20, 2, ROWS)).astype(F8)
        eps = Xl[:, 0]
        E9 = np.empty((ROWS, 9), np.float32)
        for j, t in enumerate(TS):
            E9[:, j] = bw[t] * eps / (SP * SP)
            E9[:, 3 + j] = Xl[:, RIDX[t]] + bb[t]
            E9[:, 6 + j] = (bw[t] * beb[t] + (bw[t] * K[t]) * eps) / SP
        XRBc = np.ascontiguousarray(
            E9.reshape(NCHUNK, 4, 128, 9).transpose(2, 0, 1, 3))
        in_maps.append({
            "XT": XTc, "XRB": XRBc, "W1T": W1T, "W2H": W2H, "CB": CB,
        })
    return in_maps


def _gather_output(results):
    Y = np.empty((B, 3), np.float32)
    for c in range(NCORES):
        Ydev = np.asarray(results[c]["Y"], np.float32)   # [128, 64, 4, 3]
        Y[c * ROWS:(c + 1) * ROWS] = (
            Ydev.transpose(1, 2, 0, 3).reshape(ROWS, 3))
    return Y


def run(inputs, trace=False, **spmd_kwargs):
    from concourse import bass_utils

    nc = _get_nc()
    in_maps = _prepare_inputs(inputs)
    res = bass_utils.run_bass_kernel_spmd(
        nc, in_maps, list(range(NCORES)), trace=trace, **spmd_kwargs)
    return _gather_output(res.results), res


def kernel(**inputs):
    out, _ = run(inputs)
    return out



# revision 4
# speedup vs baseline: 2.0500x; 2.0500x over previous
"""TRN2 Bass kernel for nn_NNModelEx_63513976373928.

Math (per row x of X [B, 38]):
  h1  = relu(x @ W1.T + b1)                  [256]
  h2  = relu(h1 @ W2.T + b2)                 [256]
  out = h2 @ W3.T + b3                       [128]
  per target t in (incl, ecc, mm), ridx in (7, 9, 12):
    lin = out . lw_t + lb_t
    e   = (out . bew_t) * eps + beb_t        eps = x[0]
    y_t = bw_t * e * lin + bb_t + x[ridx]

Device strategy (pure data parallel, 8 cores x 32768 rows):
  - feature-on-partition layout: H1T/H2T [units, rows], rows chunked by 512
  - b1 folded into the L1 matmul via an augmented contraction row
    (XT row 38 = 1, W1T row 38 = b1) -> single merged L1 relu cast
  - L3 + heads folded: lin/p are dots of h2 with W3.T @ lw / W3.T @ bew
    (6 vectors precomputed on host, b3 contributions folded into consts)
  - X pre-transposed and fp8-cast on host -> XT [40, 32768] per core
  - residual/eps/bias columns packed fp32 on host -> XRB [128, 64, 4, 9]
  - head elementwise split DVE (PSUM reads) / Pool (SBUF-only ops)

Matmul flavors tuned for the HW weight-load path (measured: per-PE-
instruction ~30ns + exposed LDWEIGHTS time dominate over raw matmul
cycles): L1 non-DR (k=40, FWL weight loads), L2 DoubleRow (k=256, its
213ns weight load hides under the preceding big matmul), heads non-DR
k=128 accumulate pairs (FWL loads hide under the tiny FD=6 matmuls).
PSUM: h1p 2x2 banks, h2pa double-buffered (2), h2pb 1, hp 1.
ACT-table preloaded at start; batch-0 X staged per-chunk.
"""

import sys

for _p in ("/opt/trn_rl_repo", "/opt/trn_rl_repo/concourse"):
    if _p not in sys.path:
        sys.path.insert(0, _p)

import numpy as np
import ml_dtypes

BF16 = ml_dtypes.bfloat16

NCORES = 8
B = 262144
D = 38
DA = 39                     # contraction with bias row appended
ROWS = B // NCORES          # 32768 rows per core
CHUNK = 512                 # rows per chunk
NCHUNK = ROWS // CHUNK      # 64
CPB = 8                     # chunks per staging batch
NBATCH = NCHUNK // CPB      # 8

_NC_CACHE = {}


def _build_nc(repeat=1):
    from concourse import bass, bacc, tile
    from contextlib import nullcontext

    mybir = bass.mybir
    f32 = mybir.dt.float32
    f8 = mybir.dt.float8e4

    nc = bacc.Bacc(None, target_bir_lowering=False, debug=False)

    XT = nc.dram_tensor("XT", [40, ROWS], f8, kind="ExternalInput")
    XRB = nc.dram_tensor("XRB", [128, NCHUNK, 4, 9], f32, kind="ExternalInput")
    W1T = nc.dram_tensor("W1T", [40, 256], f8, kind="ExternalInput")
    # W2H packs W2 (cols 0:256, two output halves) + head vectors (256:262);
    # padded to 272 so the dim-1 stride is 16B-aligned (dual-row fp8 ISA rule)
    W2H = nc.dram_tensor("W2H", [128, 2, 272], f8, kind="ExternalInput")
    # CB packs lin consts ([:, 0:4, :]) + b2 halves ([:, 4, 0:2])
    CB = nc.dram_tensor("CB", [128, 5, 3], f32, kind="ExternalInput")
    Y = nc.dram_tensor("Y", [128, NCHUNK, 4, 3], f32, kind="ExternalOutput")

    with tile.TileContext(nc) as tc:
        with (
            tc.tile_pool(name="wpool", bufs=1) as wpool,
            tc.tile_pool(name="xpool", bufs=2) as xpool,
            tc.tile_pool(name="h1pool", bufs=3) as h1pool,
            tc.tile_pool(name="h2pool", bufs=4) as h2pool,
            tc.tile_pool(name="spool", bufs=3) as spool,
            tc.tile_pool(name="bpool", bufs=3) as bpool,
            tc.tile_pool(name="ypool", bufs=3) as ypool,
            tc.tile_pool(name="psl1", bufs=2, space="PSUM") as psl1,
            tc.tile_pool(name="psl2", bufs=1, space="PSUM") as psl2,
            tc.tile_pool(name="pshead", bufs=2, space="PSUM") as pshead,
        ):
            w1t = wpool.tile([40, 256], f8)
            nc.sync.dma_start(w1t[:], W1T[:])
            w2h = wpool.tile([128, 2, 272], f8)
            nc.sync.dma_start(w2h[:], W2H[:])
            cb = wpool.tile([128, 5, 3], f32)
            nc.sync.dma_start(cb[:], CB[:])
            # preload the ACT function table off the critical path
            scr = wpool.tile([128, 1], mybir.dt.float32)
            nc.vector.memset(scr[:], 0.0)
            Relu = mybir.ActivationFunctionType.Relu
            nc.scalar.activation(scr[:], scr[:], Relu, bias=0.0, scale=1.0)

            rep_ctx = tc.For_i(0, repeat) if repeat > 1 else nullcontext()
            with rep_ctx:
                _kernel_body(nc, tc, locals())

    nc.finalize()
    return nc


def _kernel_body(nc, tc, env):
    from concourse import bass

    mybir = bass.mybir
    f32 = mybir.dt.float32
    f8 = mybir.dt.float8e4
    DR = mybir.MatmulPerfMode.DoubleRow
    Relu = mybir.ActivationFunctionType.Relu
    add = mybir.AluOpType.add
    mult = mybir.AluOpType.mult
    amax = mybir.AluOpType.max
    TT = nc.vector.tensor_tensor
    PTT = nc.gpsimd.tensor_tensor
    XT, XRB, Y = env["XT"], env["XRB"], env["Y"]
    w1t, w2h, cb = env["w1t"], env["w2h"], env["cb"]
    xpool, h1pool, h2pool, spool, bpool = (
        env["xpool"], env["h1pool"], env["h2pool"], env["spool"],
        env["bpool"])
    psl1, psl2, pshead = env["psl1"], env["psl2"], env["pshead"]

    ypool = env["ypool"]
    xrb_t = [None] * NBATCH
    xt_t = [None] * NBATCH
    hp_t = [None] * NBATCH
    h1_t = [None] * NCHUNK
    h2_t = [None] * NCHUNK

    def stage_in(bi):
        base = bi * CPB * CHUNK
        xt = xpool.tile([40, CPB * CHUNK], f8, name="xt", bufs=2)
        if bi == 0:
            # per-chunk staging so chunk c's L1 starts as soon as its own
            # 512 rows land (shortens the pipeline fill)
            for c in range(CPB):
                lo, hi = c * CHUNK, (c + 1) * CHUNK
                nc.sync.dma_start(xt[:, lo:hi],
                                  XT[:, base + lo:base + hi])
        else:
            nc.sync.dma_start(
                xt[:], XT[:, base:base + CPB * CHUNK])
        xt_t[bi] = xt
        xrb = bpool.tile([128, CPB, 4, 9], f32, name="xrb", bufs=3)
        nc.sync.dma_start(xrb[:], XRB[:, bi * CPB:(bi + 1) * CPB, :, :])
        xrb_t[bi] = xrb

    def epilogue(bi, off, n, suf):
        # y = ((p*bweps + ebias) * (lin + lb')) + (xr + bb)
        #   xrb cols: 0:3 bw*eps/SP^2, 3:6 xr+bb, 6:9 ebias/SP
        hp = hp_t[bi]
        xrb = xrb_t[bi]
        hs = slice(off, off + n)
        cb_lin = cb[:, None, 0:4, :].to_broadcast([128, n, 4, 3])
        linp = spool.tile([128, n, 4, 3], f32, name="linp" + suf, bufs=3)
        e = spool.tile([128, n, 4, 3], f32, name="e" + suf, bufs=3)
        ystg = ypool.tile([128, n, 4, 3], f32, name="ystg" + suf, bufs=3)
        TT(out=linp[:], in0=hp[:, hs, :, 0:3], in1=cb_lin, op=add)
        TT(out=e[:], in0=hp[:, hs, :, 3:6], in1=xrb[:, hs, :, 0:3],
           op=mult)
        PTT(out=e[:], in0=e[:], in1=xrb[:, hs, :, 6:9], op=add)
        PTT(out=e[:], in0=e[:], in1=linp[:], op=mult)
        PTT(out=ystg[:], in0=e[:], in1=xrb[:, hs, :, 3:6], op=add)
        nc.sync.dma_start(
            Y[:, bi * CPB + off:bi * CPB + off + n, :, :], ystg[:])

    # 4-stage software pipeline over chunks; iteration ci emits on PE
    # [L1a(ci), L2a(ci-1), L1b(ci), L2b(ci-1), heads(ci-3)].
    # Matmul flavors chosen for the HW weight-load path: L1 non-DR (k=40,
    # FWL ~27ns loads), L2 DR (k=256; its 213ns DR load hides under the
    # preceding L1/L2 matmul), heads non-DR k=128 accumulate pairs (FWL
    # loads hide under the tiny head matmuls). PSUM budget (8 banks):
    # h1p 2x2 + h2pa 1 + h2pb 1 + hp 2x1.
    stage_in(0)
    for ci in range(NCHUNK + 3):
        ck = ci - 3
        cj = ci - 1
        in_l1 = ci < NCHUNK
        in_l2 = 0 <= cj < NCHUNK
        if in_l1:
            bi, cbk = divmod(ci, CPB)
            if cbk == 0 and bi + 1 < NBATCH:
                stage_in(bi + 1)
            # L1: H1T = W1T.T @ XT, bias via augmented row
            # weights host-scaled x64 for fp8; descaled via the ACT port
            h1p = psl1.tile([128, 2, CHUNK], f32, name="h1p", bufs=2)
            xt = xt_t[bi]
            sl = slice(cbk * CHUNK, (cbk + 1) * CHUNK)
            nc.tensor.matmul(h1p[:, 0, :], w1t[:, 0:128], xt[:, sl],
                             start=True, stop=True)
        if in_l2:
            h1 = h1_t[cj]
            h1_t[cj] = None
            h2pa = psl2.tile([128, CHUNK], f32, name="h2pa", bufs=2)
            nc.tensor.matmul(h2pa[:], w2h[:, :, 0:128], h1[:],
                             start=True, stop=True, perf_mode=DR)
        if in_l1:
            nc.tensor.matmul(h1p[:, 1, :], w1t[:, 128:256], xt[:, sl],
                             start=True, stop=True)
        if in_l2:
            h2pb = psl2.tile([128, CHUNK], f32, name="h2pb", bufs=1)
            nc.tensor.matmul(h2pb[:], w2h[:, :, 128:256], h1[:],
                             start=True, stop=True, perf_mode=DR)
        if ck >= 0:
            bi2, cbk2 = divmod(ck, CPB)
            if cbk2 == 0:
                hp_t[bi2] = pshead.tile([128, CPB, 4, 6], f32, name="hp",
                                        bufs=1)
            hp = hp_t[bi2]
            h2ab = h2_t[ck]
            h2_t[ck] = None
            # heads: hp[:, cbk, s, 0:3] = lin_mm, hp[:, cbk, s, 3:6] = p_mm
            for s in range(4):
                seg = slice(s * 128, (s + 1) * 128)
                nc.tensor.matmul(hp[:, cbk2, s, :], h2ab[:, 0, seg],
                                 w2h[:, 0, 256:262], start=True, stop=False)
                nc.tensor.matmul(hp[:, cbk2, s, :], h2ab[:, 1, seg],
                                 w2h[:, 1, 256:262], start=False, stop=True)

        if in_l1:
            h1 = h1pool.tile([128, 2, CHUNK], f8, name="h1", bufs=3)
            nc.scalar.activation(h1[:], h1p[:], Relu, bias=0.0,
                                 scale=1.0 / 64)
            h1_t[ci] = h1

        if in_l2:
            # W2/B2 host-scaled x16 -> casts produce 16*h2 in fp8
            # (head vectors carry the matching descale)
            h2ab = h2pool.tile([128, 2, CHUNK], f8, name="h2ab", bufs=4)
            # a always on DVE (first, so its PSUM slot frees early);
            # b moves to ACT 1 chunk in 4 to balance engine load
            nc.vector.tensor_scalar(h2ab[:, 0, :], h2pa[:], cb[:, 4, 0:1],
                                    0.0, op0=add, op1=amax)
            if cj % 4 == 1:
                nc.scalar.activation(h2ab[:, 1, :], h2pb[:], Relu,
                                     bias=cb[:, 4, 1:2], scale=1.0)
            else:
                nc.vector.tensor_scalar(h2ab[:, 1, :], h2pb[:],
                                        cb[:, 4, 1:2], 0.0,
                                        op0=add, op1=amax)
            h2_t[cj] = h2ab

        # epilogue in the same iteration as the batch's last heads (hp is
        # single-buffered); the final batch is split 4+2+2 to shorten the
        # pipeline drain
        if ck >= 0:
            if ck == NCHUNK - 5:
                epilogue(NBATCH - 1, 0, 4, "q")
            elif ck == NCHUNK - 3:
                epilogue(NBATCH - 1, 4, 2, "r")
            elif ck == NCHUNK - 1:
                epilogue(NBATCH - 1, 6, 2, "r")
            elif ck % CPB == CPB - 1:
                epilogue(ck // CPB, 0, CPB, "")


def _get_nc():
    if "nc" not in _NC_CACHE:
        _NC_CACHE["nc"] = _build_nc()
    return _NC_CACHE["nc"]


def _prepare_inputs(inputs):
    X = np.asarray(inputs["X"], dtype=np.float32)
    W1 = np.asarray(inputs["W1"], dtype=np.float32)
    b1 = np.asarray(inputs["b1"], dtype=np.float32)
    W2 = np.asarray(inputs["W2"], dtype=np.float32)
    b2 = np.asarray(inputs["b2"], dtype=np.float32)
    W3 = np.asarray(inputs["W3"], dtype=np.float32)
    b3 = np.asarray(inputs["b3"], dtype=np.float32)

    lw, lb, bew, beb, bw, bb = {}, {}, {}, {}, {}, {}
    for t in ("incl", "ecc", "mm"):
        lw[t] = np.asarray(inputs[f"lin_w_{t}"], np.float32)[0]        # [128]
        lb[t] = float(np.asarray(inputs[f"lin_b_{t}"], np.float32)[0])
        bew[t] = np.asarray(inputs[f"bile_w_{t}"], np.float32)[0][:, 0]  # [128]
        beb[t] = float(np.asarray(inputs[f"bile_b_{t}"], np.float32)[0])
        bw[t] = float(np.asarray(inputs[f"bil_w_{t}"], np.float32)[0, 0, 0])
        bb[t] = float(np.asarray(inputs[f"bil_b_{t}"], np.float32)[0])
    TS = ("incl", "ecc", "mm")
    RIDX = {"incl": 7, "ecc": 9, "mm": 12}

    # ---- replicated weights (fp8 DoubleRow layouts) ----
    # scales: W1 x64 (descaled in h1 ACT cast), W2/B2 x16 (h2 lives at
    # 16x in fp8, max |h2|<15 assumed), HW2 x16 -> hp at 256x; the 1/256
    # descale is folded exactly (powers of 2) into CONSTS/XRB.
    F8 = ml_dtypes.float8_e4m3
    SC1, SC2, SCH = 64.0, 16.0, 16.0
    SP = SC2 * SCH                                                  # 256
    W1a = np.zeros((40, 256), np.float32)
    W1a[0:D] = W1.T * SC1
    W1a[D] = b1 * SC1
    W1T = W1a.astype(F8)                                            # [40, 256]
    O6 = np.stack([lw[t] for t in TS] + [bew[t] for t in TS], axis=1)  # [128,6]
    HW2f = W3.T.astype(np.float32) @ O6                             # [256, 6]
    W2H = np.zeros((128, 2, 272), np.float32)
    # cols 0:256: W2.T [k, m] at [k % 128, k // 128, oh*128 + m], x16
    W2H[:, :, 0:256] = (
        W2.T.reshape(2, 128, 256).transpose(1, 0, 2)) * SC2
    W2H[:, :, 256:262] = (
        HW2f.reshape(2, 128, 6).transpose(1, 0, 2)) * SCH
    W2H = W2H.astype(F8)
    c3 = np.array(
        [lb[t] + float(b3 @ lw[t]) for t in TS],         # lb' (b3 folded)
        dtype=np.float32) * SP
    CB = np.empty((128, 5, 3), np.float32)
    CB[:, 0:4, :] = c3
    CB[:, 4, 0] = b2[0:128] * SC2
    CB[:, 4, 1] = b2[128:256] * SC2
    CB[:, 4, 2] = 0.0
    K = {t: float(b3 @ bew[t]) for t in TS}

    in_maps = []
    for c in range(NCORES):
        Xl = X[c * ROWS:(c + 1) * ROWS]                             # [32768, 38]
        XTf = np.zeros((40, ROWS), np.float32)
        XTf[0:D] = Xl.T
        XTf[D] = 1.0
        XTc = XTf.astype(F8)                                        # [40, ROWS]
        eps = Xl[:, 0]
        E9 = np.empty((ROWS, 9), np.float32)
        for j, t in enumerate(TS):
            E9[:, j] = bw[t] * eps / (SP * SP)
            E9[:, 3 + j] = Xl[:, RIDX[t]] + bb[t]
            E9[:, 6 + j] = (bw[t] * beb[t] + (bw[t] * K[t]) * eps) / SP
        XRBc = np.ascontiguousarray(
            E9.reshape(NCHUNK, 4, 128, 9).transpose(2, 0, 1, 3))
        in_maps.append({
            "XT": XTc, "XRB": XRBc, "W1T": W1T, "W2H": W2H, "CB": CB,
        })
    return in_maps


def _gather_output(results):
    Y = np.empty((B, 3), np.float32)
    for c in range(NCORES):
        Ydev = np.asarray(results[c]["Y"], np.float32)   # [128, 64, 4, 3]
        Y[c * ROWS:(c + 1) * ROWS] = (
            Ydev.transpose(1, 2, 0, 3).reshape(ROWS, 3))
    return Y


def run(inputs, trace=False, **spmd_kwargs):
    from concourse import bass_utils

    nc = _get_nc()
    in_maps = _prepare_inputs(inputs)
    res = bass_utils.run_bass_kernel_spmd(
        nc, in_maps, list(range(NCORES)), trace=trace, **spmd_kwargs)
    return _gather_output(res.results), res


def kernel(**inputs):
    out, _ = run(inputs)
    return out

